# revision 29
# baseline (speedup 1.0000x reference)
"""Trainium2 Bass kernel for nn_DecoderLayer_90074054132191.

Sharding: 8 cores = 2 batch groups x 4 cores. Core c: batch b=c//4, head
group g=c%4 (heads 4g..4g+4), token slice g*256..(g+1)*256 after a
ReduceScatter of the partial attention output over the 4-core group.

Phase A is software-pipelined over the two 512-token halves: each gate
stage runs for half h0 as soon as its inputs exist, so the first
ReduceScatter fires while half h1 is still in the gate pipeline, and the
FFN (phase B, split per 128-token output block) overlaps the second
ReduceScatter.
"""

import math
from contextlib import ExitStack

import ml_dtypes
import numpy as np

import concourse.bass as bass
import concourse.mybir as mybir
import concourse.tile as tile
from concourse import bacc
from concourse.bass_utils import run_bass_kernel_spmd

F32 = mybir.dt.float32
BF16 = mybir.dt.bfloat16
AF = mybir.ActivationFunctionType
OP = mybir.AluOpType

B, L, D, H, F = 2, 1024, 1024, 16, 4096
DH = 64          # head dim
HPC = 4          # heads per core
DC = HPC * DH    # 256 features per core
TOK = 256        # tokens per core after reduce-scatter
C = 128          # chunk size
NCH = L // C     # 8 chunks
NKT = D // 128   # 8 k-tiles of the model dim


def build_program(sim_nocc=False):
    nc = bacc.Bacc("TRN2", target_bir_lowering=False, num_devices=8)

    # ---- external I/O (per-core shards prepared on host) ----
    xT = nc.dram_tensor("xT", [D, L], BF16, kind="ExternalInput")
    wq = nc.dram_tensor("wq", [D, DC], BF16, kind="ExternalInput")
    wk = nc.dram_tensor("wk", [D, DC], BF16, kind="ExternalInput")
    wv = nc.dram_tensor("wv", [D, DC], BF16, kind="ExternalInput")
    wo = nc.dram_tensor("wo", [DC, D], BF16, kind="ExternalInput")
    w1 = nc.dram_tensor("w1", [D, F], BF16, kind="ExternalInput")
    w2 = nc.dram_tensor("w2", [F, D], BF16, kind="ExternalInput")
    xres = nc.dram_tensor("xres", [TOK, D], F32, kind="ExternalInput")
    triu_c = nc.dram_tensor("triu", [C, 4 * C], BF16, kind="ExternalInput")
    ident_c = nc.dram_tensor("ident", [128, 128], BF16, kind="ExternalInput")
    bd_c = nc.dram_tensor("bd", [128, 144], BF16, kind="ExternalInput")
    ohp_c = nc.dram_tensor("ohp", [36, 512], BF16, kind="ExternalInput")
    nrm_c = nc.dram_tensor("nrm", [36, L], F32, kind="ExternalInput")
    inrm_c = nc.dram_tensor("inrm", [36, L], F32, kind="ExternalInput")
    out = nc.dram_tensor("out", [TOK, D], F32, kind="ExternalOutput")

    with ExitStack() as top:
        tc = top.enter_context(tile.TileContext(nc))
        consts = top.enter_context(tc.tile_pool(name="consts", bufs=1))
        dram = top.enter_context(tc.tile_pool(name="dram", bufs=1, space="DRAM"))

        rs_in = dram.tile([L, D], BF16)
        rs_out0 = dram.tile([TOK // 2, D], BF16)
        rs_out1 = dram.tile([TOK // 2, D], BF16)

        # ---- consts to SBUF ----
        triu = consts.tile([C, 4 * C], BF16)
        ident = consts.tile([128, 128], BF16)
        bd = consts.tile([128, 144], BF16)
        ohp = consts.tile([36, 512], BF16)
        nrm = consts.tile([36, L], F32)
        inrm = consts.tile([36, L], F32)
        epsb = consts.tile([128, 1], F32)
        nc.vector.memset(epsb, 1e-5)
        scr1 = consts.tile([1, 1], F32)  # ACT-table preload scratch

        w1pool = top.enter_context(tc.tile_pool(name="w1pool", bufs=1))
        w1_sb = []
        early = top.enter_context(tc.tile_pool(name="early", bufs=1))
        hT_sb = []
        hn0 = None
        with ExitStack() as pa:  # ---------------- PHASE A ----------------
            awork = pa.enter_context(tc.tile_pool(name="awork", bufs=1))
            pbig = pa.enter_context(
                tc.tile_pool(name="pbig", bufs=2, space="PSUM"))
            psmall = pa.enter_context(
                tc.tile_pool(name="psmall", bufs=2, space="PSUM"))
            pyp = pa.enter_context(tc.tile_pool(name="pyp", bufs=2, space="PSUM"))
            pkv = pa.enter_context(tc.tile_pool(name="pkv", bufs=1, space="PSUM"))

            xw = ExitStack()
            xwpool = xw.enter_context(tc.tile_pool(name="xwpool", bufs=1))
            xT_sb, wq_sb, wk_sb, wv_sb = [], [], [], []
            for kt in range(NKT):
                t = xwpool.tile([128, L], BF16, name=f"xT{kt}")
                nc.sync.dma_start(t[:, 0:512], xT[kt * 128:(kt + 1) * 128, 0:512])
                xT_sb.append(t)
                t2 = xwpool.tile([128, DC], BF16, name=f"wk{kt}")
                nc.sync.dma_start(t2, wk[kt * 128:(kt + 1) * 128, :])
                wk_sb.append(t2)
            for kt in range(NKT):
                t2 = xwpool.tile([128, DC], BF16, name=f"wq{kt}")
                nc.sync.dma_start(t2, wq[kt * 128:(kt + 1) * 128, :])
                wq_sb.append(t2)
            last_load = None
            for kt in range(NKT):
                nc.sync.dma_start(xT_sb[kt][:, 512:1024],
                                  xT[kt * 128:(kt + 1) * 128, 512:1024])
                t2 = xwpool.tile([128, DC], BF16, name=f"wv{kt}")
                last_load = nc.sync.dma_start(t2, wv[kt * 128:(kt + 1) * 128, :])
                wv_sb.append(t2)
            nc.sync.dma_start(bd, bd_c[:])
            nc.sync.dma_start(nrm, nrm_c[:])
            nc.sync.dma_start(inrm, inrm_c[:])
            nc.sync.dma_start(triu, triu_c[:])
            nc.sync.dma_start(ident, ident_c[:])
            nc.sync.dma_start(ohp, ohp_c[:])
            wo_sb = []
            for p in range(2):
                t = awork.tile([128, D], BF16, name=f"wo{p}")
                nc.sync.dma_start(t, wo[p * 128:(p + 1) * 128, :])
                wo_sb.append(t)
            # deferred weight prefetches (behind the projection loads)
            for kt in range(NKT):
                t = w1pool.tile([128, F], BF16, name=f"w1{kt}")
                d = nc.sync.dma_start(t, w1[kt * 128:(kt + 1) * 128, :])
                tile.add_dep_helper(d.ins, last_load.ins, sync=False,
                                    reason="defer w1 prefetch")
                w1_sb.append(t)

            # ---- persistent tiles ----
            Q = [awork.tile([128, L], BF16, name=f"q{p}") for p in range(2)]
            K = [awork.tile([128, L], BF16, name=f"k{p}") for p in range(2)]
            V = [awork.tile([128, L], BF16, name=f"v{p}") for p in range(2)]
            # scan outputs (also reused for stage-2 scans)
            ck = [awork.tile([128, L], BF16, name=f"ck{p}") for p in range(2)]
            cq = [awork.tile([128, L], BF16, name=f"cq{p}") for p in range(2)]
            # mul products feeding the bd matmuls
            m1 = [awork.tile([128, L], BF16, name=f"m1{p}") for p in range(2)]
            m2 = [awork.tile([128, L], BF16, name=f"m2{p}") for p in range(2)]
            # stage-2 products
            wk2 = [awork.tile([128, L], BF16, name=f"wk2{p}") for p in range(2)]
            wq2 = [awork.tile([128, L], BF16, name=f"wq2{p}") for p in range(2)]
            DEN = awork.tile([36, L], F32)
            IDEN = awork.tile([36, L], F32)
            S = awork.tile([36, L], BF16)
            CONS = awork.tile([36, L], F32)
            R2 = awork.tile([36, L], BF16)
            nc.gpsimd.memset(R2, 0.0)
            KT_sb = [awork.tile([128, L], BF16, name=f"ktok{p}")
                     for p in range(2)]
            VT_sb = [awork.tile([128, L], BF16, name=f"vtok{p}")
                     for p in range(2)]
            qfacB = [awork.tile([128, L], BF16, name=f"qfacB{p}")
                     for p in range(2)]
            attnT = [awork.tile([128, L], BF16, name=f"attnT{p}")
                     for p in range(2)]
            kv_sb = [awork.tile([128, DH], BF16, name=f"kv{p}")
                     for p in range(2)]
            kvps = [pkv.tile([128, DH], F32, name=f"kvps{p}", tag=f"kvp{p}")
                    for p in range(2)]
            M2 = {}

            def proj(h):
                sl = slice(h * 512, (h + 1) * 512)
                for nm, wsb, dst in (("k", wk_sb, K), ("q", wq_sb, Q),
                                     ("v", wv_sb, V)):
                    for mt in range(2):
                        pool, tg = [(pbig, "big"), (pyp, "y"),
                                    (psmall, "sm")][mt % 3]
                        ps = pool.tile([128, 512], F32, tag=tg)
                        for kt in range(NKT):
                            nc.tensor.matmul(
                                ps,
                                wsb[kt][:, mt * 128:(mt + 1) * 128],
                                xT_sb[kt][:, sl],
                                start=(kt == 0), stop=(kt == NKT - 1))
                        dsl = dst[mt][:, sl]
                        if nm == "v":
                            nc.scalar.copy(out=dsl, in_=ps)
                        else:
                            nc.scalar.activation(dsl, ps, AF.Sigmoid)

            def scan1(h):
                sl = slice(h * 512, (h + 1) * 512)
                for p in range(2):
                    ini_k = 0.0 if h == 0 else ck[p][:, 511:512]
                    ini_q = 0.0 if h == 0 else cq[p][:, 511:512]
                    nc.vector.tensor_tensor_scan(ck[p][:, sl], K[p][:, sl],
                                                 K[p][:, sl], ini_k,
                                                 OP.add, OP.bypass)
                    nc.vector.tensor_tensor_scan(cq[p][:, sl], Q[p][:, sl],
                                                 Q[p][:, sl], ini_q,
                                                 OP.add, OP.bypass)

            def mmul(h):  # m1 = Q*cumK, m2 = K*cumQ
                sl = slice(h * 512, (h + 1) * 512)
                for p in range(2):
                    eng = nc.vector if p == 0 else nc.gpsimd
                    eng.tensor_mul(out=m1[p][:, sl], in0=Q[p][:, sl],
                                   in1=ck[p][:, sl])
                    eng.tensor_mul(out=m2[p][:, sl], in0=K[p][:, sl],
                                   in1=cq[p][:, sl])

            def bdmm(h, srcs, dst_f32, via):
                sl = slice(h * 512, (h + 1) * 512)
                ps = pbig.tile([36, 512], F32, tag="big")
                nc.tensor.matmul(ps, bd[:, 0:36], srcs[0][0][:, sl],
                                 start=True, stop=False)
                nc.tensor.matmul(ps, bd[:, 36:72], srcs[0][1][:, sl],
                                 start=False, stop=False)
                nc.tensor.matmul(ps, bd[:, 72:108], srcs[1][0][:, sl],
                                 start=False, stop=False)
                nc.tensor.matmul(ps, bd[:, 108:144], srcs[1][1][:, sl],
                                 start=False, stop=True)
                if via == "vadd":
                    # +1e-30 keeps unused zero rows finite through reciprocal
                    nc.vector.tensor_scalar_add(out=dst_f32[:, sl], in0=ps,
                                                scalar1=1e-30)
                else:
                    nc.scalar.copy(out=dst_f32[:, sl], in_=ps)

            def ktm2(h):
                for p in range(2):
                    for ci in range(4):
                        c = h * 4 + ci
                        sl = slice(c * 128, (c + 1) * 128)
                        pst = psmall.tile([128, 128], BF16, tag="sm")
                        nc.tensor.transpose(pst, K[p][:, sl], ident)
                        nc.scalar.copy(out=KT_sb[p][:, sl], in_=pst)
                # packed masked diag blocks: one [128,512] psum per (p,hh)
                for p in range(2):
                    for hh in range(2):
                        rows = slice(hh * 64, (hh + 1) * 64)
                        aps = pyp.tile([128, 512], F32, tag="y")
                        for ci in range(4):
                            c = h * 4 + ci
                            sl = slice(c * 128, (c + 1) * 128)
                            nc.tensor.matmul(aps[:, ci * 128:(ci + 1) * 128],
                                             K[p][rows, sl], Q[p][rows, sl],
                                             start=True, stop=True)
                        m = awork.tile([128, 512], BF16,
                                       name=f"m2_{p}_{h}_{hh}")
                        nc.vector.tensor_mul(out=m, in0=aps, in1=triu)
                        for ci in range(4):
                            M2[(p, h * 4 + ci, hh)] = \
                                m[:, ci * 128:(ci + 1) * 128]

            def denpost(h):
                sl = slice(h * 512, (h + 1) * 512)
                nc.vector.reciprocal_approx_fast(out=IDEN[:, sl],
                                                 in_=DEN[:, sl])
                nc.vector.tensor_mul(out=S[:, sl], in0=nrm[:, sl],
                                     in1=IDEN[:, sl])

            def ohps(h):  # broadcast S rows; wk2 = K*src_out, wq2 = Q*sink_in
                sl = slice(h * 512, (h + 1) * 512)
                for p in range(2):
                    ps1 = pbig.tile([128, 512], F32, tag="big")
                    ps2 = pbig.tile([128, 512], F32, tag="big")
                    nc.tensor.matmul(ps1, ohp[:, 256 + p * 128:256 + (p + 1) * 128],
                                     S[:, sl], start=True, stop=True)
                    nc.tensor.matmul(ps2, ohp[:, p * 128:(p + 1) * 128],
                                     S[:, sl], start=True, stop=True)
                    nc.vector.tensor_mul(out=wk2[p][:, sl], in0=K[p][:, sl],
                                         in1=ps1)
                    nc.vector.tensor_mul(out=wq2[p][:, sl], in0=Q[p][:, sl],
                                         in1=ps2)

            def scan2(h):  # cumsum of wk2/wq2 into ck/cq (scan1 values dead)
                sl = slice(h * 512, (h + 1) * 512)
                for p in range(2):
                    ini_k = 0.0 if h == 0 else ck[p][:, 511:512]
                    ini_q = 0.0 if h == 0 else cq[p][:, 511:512]
                    nc.vector.tensor_tensor_scan(ck[p][:, sl], wk2[p][:, sl],
                                                 wk2[p][:, sl], ini_k,
                                                 OP.add, OP.bypass)
                    nc.vector.tensor_tensor_scan(cq[p][:, sl], wq2[p][:, sl],
                                                 wq2[p][:, sl], ini_q,
                                                 OP.add, OP.bypass)

            def mmul2(h):  # m1 = Q*cum(K*src), m2 = K*cum(Q*sink)
                sl = slice(h * 512, (h + 1) * 512)
                for p in range(2):
                    eng = nc.vector if p == 0 else nc.gpsimd
                    eng.tensor_mul(out=m1[p][:, sl], in0=Q[p][:, sl],
                                   in1=ck[p][:, sl])
                    eng.tensor_mul(out=m2[p][:, sl], in0=K[p][:, sl],
                                   in1=cq[p][:, sl])

            def conspost(h):
                sl = slice(h * 512, (h + 1) * 512)
                nc.vector.tensor_mul(out=CONS[:, sl], in0=CONS[:, sl],
                                     in1=inrm[:, sl])
                nc.vector.tensor_scalar(out=CONS[0:4, sl], in0=CONS[0:4, sl],
                                        scalar1=1.0, scalar2=-1.0,
                                        op0=OP.min, op1=OP.max)
                EX = S  # reuse
                nc.scalar.activation(EX[32:36, sl], CONS[32:36, sl], AF.Sigmoid)
                nc.vector.tensor_mul(out=R2[32:36, sl], in0=IDEN[32:36, sl],
                                     in1=EX[32:36, sl])
                nc.scalar.activation(EX[0:4, sl], CONS[0:4, sl], AF.Exp)
                CE = CONS
                ini = 0.0 if h == 0 else CE[0:4, 511:512]
                nc.vector.tensor_tensor_scan(CE[0:4, sl], EX[0:4, sl],
                                             EX[0:4, sl], ini,
                                             OP.add, OP.bypass)
                nc.vector.reciprocal_approx_fast(out=IDEN[0:4, sl],
                                                 in_=CE[0:4, sl])
                nc.vector.tensor_mul(out=EX[0:4, sl], in0=EX[0:4, sl],
                                     in1=IDEN[0:4, sl])
                nc.vector.tensor_mul(out=R2[0:4, sl], in0=EX[0:4, sl],
                                     in1=nrm[0:4, sl])

            def qfac(h):
                sl = slice(h * 512, (h + 1) * 512)
                for p in range(2):
                    ps1 = pbig.tile([128, 512], F32, tag="big")
                    ps2 = pbig.tile([128, 512], F32, tag="big")
                    nc.tensor.matmul(ps1, ohp[:, p * 128:(p + 1) * 128],
                                     R2[:, sl], start=True, stop=True)
                    nc.tensor.matmul(ps2, ohp[:, 256 + p * 128:256 + (p + 1) * 128],
                                     R2[:, sl], start=True, stop=True)
                    nc.scalar.copy(out=qfacB[p][:, sl], in_=ps1)
                    nc.vector.tensor_mul(out=V[p][:, sl], in0=V[p][:, sl],
                                         in1=ps2)

            def vst(h):
                for p in range(2):
                    for ci in range(4):
                        c = h * 4 + ci
                        sl = slice(c * 128, (c + 1) * 128)
                        pst2 = psmall.tile([128, 128], BF16, tag="sm")
                        nc.tensor.transpose(pst2, V[p][:, sl], ident)
                        nc.scalar.copy(out=VT_sb[p][:, sl], in_=pst2)

            def attn(grp):
                yp = [pyp.tile([128, 512], F32, tag="y", name=f"yp{grp}_{p}")
                      for p in range(2)]
                for ci in range(4):
                    c = grp * 4 + ci
                    sl = slice(c * 128, (c + 1) * 128)
                    for p in range(2):
                        ysl = yp[p][:, ci * 128:(ci + 1) * 128]
                        for hh in range(2):
                            rows = slice(hh * 64, (hh + 1) * 64)
                            first = (c == 0)
                            if not first:
                                nc.tensor.matmul(ysl[rows, :], kv_sb[p][rows, :],
                                                 Q[p][rows, sl],
                                                 start=True, stop=False)
                            nc.tensor.matmul(
                                ysl[rows, :],
                                VT_sb[p][:, c * 128 + hh * 64:c * 128 + hh * 64 + 64],
                                M2[(p, c, hh)], start=first, stop=True)
                            nc.tensor.matmul(
                                kvps[p][rows, :],
                                KT_sb[p][:, c * 128 + hh * 64:c * 128 + hh * 64 + 64],
                                VT_sb[p][:, c * 128 + hh * 64:c * 128 + hh * 64 + 64],
                                start=first, stop=(c == NCH - 1))
                    if c < NCH - 1:
                        for p in range(2):
                            nc.vector.tensor_copy(out=kv_sb[p], in_=kvps[p])
                for p in range(2):
                    nc.vector.tensor_mul(
                        out=attnT[p][:, grp * 512:(grp + 1) * 512],
                        in0=qfacB[p][:, grp * 512:(grp + 1) * 512], in1=yp[p])

            def womm(grp):
                for tci in range(4):
                    tch = grp * 4 + tci
                    wo_out = awork.tile([128, D], BF16, tag="wo_out", bufs=2)
                    for nt in range(2):
                        ps = pbig.tile([128, 512], F32, tag="big")
                        for p in range(2):
                            nc.tensor.matmul(
                                ps, attnT[p][:, tch * 128:(tch + 1) * 128],
                                wo_sb[p][:, nt * 512:(nt + 1) * 512],
                                start=(p == 0), stop=(p == 1))
                        osl = wo_out[:, nt * 512:(nt + 1) * 512]
                        if nt == 0:
                            nc.scalar.copy(out=osl, in_=ps)
                        else:
                            nc.vector.tensor_copy(out=osl, in_=ps)
                    nc.sync.dma_start(rs_in[tch * 128:(tch + 1) * 128, :], wo_out)

            def rstrig(grp):
                rs_half = rs_out0 if grp == 0 else rs_out1
                if sim_nocc:
                    nc.sync.dma_start(rs_half[:, :],
                                      rs_in[grp * 512:grp * 512 + TOK // 2, :])
                else:
                    nc.gpsimd.collective_compute(
                        "ReduceScatter", OP.add,
                        replica_groups=[[0, 1, 2, 3], [4, 5, 6, 7]],
                        ins=[rs_in[grp * 512:(grp + 1) * 512, :].opt()],
                        outs=[rs_half.opt()])

            # ---------- pipelined emission ----------
            proj(0)                 # PE: h0 projections
            proj(1)                 # PE: h1 projections
            xw.close()              # free xT + qkv weight SBUF
            scan1(0)                # DVE p0 / GpSimd p1
            mmul(0)                 # DVE p0 / GpSimd p1
            bdmm(0, (m1, m2), DEN, "vadd")       # PE + scalar copy(+1e-30)
            # preload the exp ACT table while the PE runs the DEN matmuls
            nc.scalar.activation(scr1, epsb[0:1, 0:1], AF.Exp)
            denpost(0)              # DVE recip + S
            scan1(1)
            ktm2(0)                 # PE transposes + packed M2 + DVE muls
            ohps(0)                 # PE + DVE
            mmul(1)
            scan2(0)
            bdmm(1, (m1, m2), DEN, "vadd")
            mmul2(0)
            denpost(1)
            bdmm(0, (m1, m2), CONS, "scopy")
            conspost(0)
            ohps(1)
            qfac(0)
            scan2(1)
            ktm2(1)
            vst(0)
            mmul2(1)
            attn(0)
            bdmm(1, (m1, m2), CONS, "scopy")
            womm(0)
            rstrig(0)
            conspost(1)
            qfac(1)
            vst(1)
            attn(1)
            womm(1)
            rstrig(1)
            # preload the sqrt ACT table before LN1 needs it
            nc.scalar.activation(scr1, epsb[0:1, 0:1], AF.Sqrt)

            # ---- tt=0 residual + LN1 + hT transposes (overlap RS#1) ----
            att0 = early.tile([128, D], BF16, name="att0")
            hh0 = early.tile([128, D], F32, name="hh0")
            att0_dma = nc.sync.dma_start(att0, rs_out0[:, :])
            nc.sync.dma_start(hh0, xres[0:128, :])
            nc.vector.tensor_add(out=hh0, in0=hh0, in1=att0)
            st0 = early.tile([128, 2, 6], F32, name="st0")
            mv0 = early.tile([128, 2], F32, name="mv0")
            for sg in range(2):
                nc.vector.bn_stats(out=st0[:, sg, :],
                                   in_=hh0[:, sg * 512:(sg + 1) * 512])
            nc.vector.bn_aggr(out=mv0, in_=st0)
            sd0 = early.tile([128, 1], F32, name="sd0")
            rstd0 = early.tile([128, 1], F32, name="rstd0")
            nc.scalar.activation(sd0, mv0[:, 1:2], AF.Sqrt, bias=epsb)
            nc.vector.reciprocal(out=rstd0, in_=sd0)
            hn0 = early.tile([128, D], BF16, name="hn0")
            nc.vector.tensor_scalar(out=hn0, in0=hh0, scalar1=mv0[:, 0:1],
                                    scalar2=rstd0, op0=OP.subtract,
                                    op1=OP.mult)
            for kt in range(NKT):
                t = early.tile([128, TOK], BF16, name=f"hT{kt}")
                hT_sb.append(t)
                pst = psmall.tile([128, 128], BF16, tag="sm")
                nc.tensor.transpose(pst, hn0[:, kt * 128:(kt + 1) * 128],
                                    ident)
                if kt % 2 == 0:
                    nc.vector.tensor_copy(out=t[:, 0:128], in_=pst)
                else:
                    nc.scalar.copy(out=t[:, 0:128], in_=pst)

        with ExitStack() as pb:  # ---------------- PHASE B ----------------
            bwork = pb.enter_context(tc.tile_pool(name="bwork", bufs=1))
            w2pool = pb.enter_context(tc.tile_pool(name="w2pool", bufs=1))
            pb1 = pb.enter_context(tc.tile_pool(name="pb1", bufs=4, space="PSUM"))
            pb2 = pb.enter_context(tc.tile_pool(name="pb2", bufs=1, space="PSUM"))

            # W2 prefetch: deferred until RS#0 completes so the collective
            # doesn't compete with an 8MB weight stream
            w2_sb = []
            for kt2 in range(F // 128):
                wt = w2pool.tile([128, D], BF16, name=f"w2{kt2}")
                d = nc.sync.dma_start(wt, w2[kt2 * 128:(kt2 + 1) * 128, :])
                tile.add_dep_helper(d.ins, att0_dma.ins, sync=False,
                                    reason="defer w2 behind RS#0")
                w2_sb.append(wt)
            att1 = bwork.tile([128, D], BF16, name="att1")
            xr1 = bwork.tile([128, D], F32, name="xr1")
            nc.sync.dma_start(att1, rs_out1[:, :])
            nc.sync.dma_start(xr1, xres[128:256, :])

            hn = [hn0, None]
            gT = [[], []]

            def ln1_tt1():  # vector-only; runs as soon as RS#1 lands
                hh_t = bwork.tile([128, D], F32, name="hh1")
                nc.vector.tensor_add(out=hh_t, in0=att1, in1=xr1)
                stats = bwork.tile([128, 2, 6], F32, tag="st", bufs=2)
                mv = bwork.tile([128, 2], F32, tag="mv", bufs=2)
                for sg in range(2):
                    nc.vector.bn_stats(out=stats[:, sg, :],
                                       in_=hh_t[:, sg * 512:(sg + 1) * 512])
                nc.vector.bn_aggr(out=mv, in_=stats)
                sd = bwork.tile([128, 1], F32, tag="sd", bufs=2)
                rstd = bwork.tile([128, 1], F32, tag="rstd", bufs=2)
                nc.scalar.activation(sd, mv[:, 1:2], AF.Sqrt, bias=epsb)
                nc.vector.reciprocal(out=rstd, in_=sd)
                hn_t = bwork.tile([128, D], BF16, name="hn1")
                nc.vector.tensor_scalar(out=hn_t, in0=hh_t,
                                        scalar1=mv[:, 0:1], scalar2=rstd,
                                        op0=OP.subtract, op1=OP.mult)
                hn[1] = hn_t

            def httrans1():
                for kt in range(NKT):
                    pst = pb1.tile([128, 128], BF16, tag="pb1")
                    nc.tensor.transpose(pst, hn[1][:, kt * 128:(kt + 1) * 128],
                                        ident)
                    nc.vector.tensor_copy(out=hT_sb[kt][:, 128:256], in_=pst)

            def w1gelu(tt):
                csl = slice(tt * 128, (tt + 1) * 128)
                for mt2 in range(F // 256):
                    ps = pb1.tile([128, 256], F32, tag="pb1")
                    for half in range(2):
                        mt = mt2 * 2 + half
                        for kt in range(NKT):
                            nc.tensor.matmul(
                                ps[:, half * 128:(half + 1) * 128],
                                w1_sb[kt][:, mt * 128:(mt + 1) * 128],
                                hT_sb[kt][:, csl], start=(kt == 0),
                                stop=(kt == NKT - 1))
                    g = bwork.tile([128, 256], BF16, name=f"g{tt}_{mt2}")
                    nc.scalar.activation(g, ps, AF.Gelu)
                    gT[tt].extend([g[:, 0:128], g[:, 128:256]])

            def w2ln(tt):
                y2ps = [pb2.tile([128, 512], F32, name=f"y2_{tt}_{nt}",
                                 tag=f"y2{nt}", bufs=2) for nt in range(2)]
                for kt2 in range(F // 128):
                    for nt in range(2):
                        nc.tensor.matmul(
                            y2ps[nt],
                            gT[tt][kt2],
                            w2_sb[kt2][:, nt * 512:(nt + 1) * 512],
                            start=(kt2 == 0), stop=(kt2 == F // 128 - 1))
                x2 = bwork.tile([128, D], F32, tag="x2", bufs=2)
                for nt in range(2):
                    nc.vector.tensor_add(
                        out=x2[:, nt * 512:(nt + 1) * 512],
                        in0=hn[tt][:, nt * 512:(nt + 1) * 512],
                        in1=y2ps[nt])
                stats = bwork.tile([128, 2, 6], F32, tag="st2", bufs=2)
                mv = bwork.tile([128, 2], F32, tag="mv2", bufs=2)
                for sg in range(2):
                    nc.vector.bn_stats(out=stats[:, sg, :],
                                       in_=x2[:, sg * 512:(sg + 1) * 512])
                nc.vector.bn_aggr(out=mv, in_=stats)
                sd2 = bwork.tile([128, 1], F32, tag="sd2", bufs=2)
                rstd = bwork.tile([128, 1], F32, tag="rstd2", bufs=2)
                nc.scalar.activation(sd2, mv[:, 1:2], AF.Sqrt, bias=epsb)
                nc.vector.reciprocal(out=rstd, in_=sd2)
                nc.vector.tensor_scalar(out=x2, in0=x2, scalar1=mv[:, 0:1],
                                        scalar2=rstd, op0=OP.subtract,
                                        op1=OP.mult)
                nc.sync.dma_start(out[tt * 128:(tt + 1) * 128, :], x2)

            w1gelu(0)
            ln1_tt1()      # hoisted: LN1 for tt=1 overlaps FFN-0 compute
            httrans1()
            w2ln(0)
            w1gelu(1)
            w2ln(1)

    nc.compile()
    return nc


_CACHE = {}
TRACE = False
LAST_RESULT = None


def _consts():
    triu = np.triu(np.ones((C, C), np.float32))
    ident = np.eye(128, dtype=np.float32)
    # row groups: src rows 0-3, sink rows 32-35 (legal partition bases)
    bd = np.zeros((128, 144), np.float32)
    for p in range(2):
        bd[0:64, p * 36 + 32 + 2 * p] = 1.0     # m1 (sink) pair p -> rows 32+
        bd[64:128, p * 36 + 32 + 2 * p + 1] = 1.0
        bd[0:64, 72 + p * 36 + 2 * p] = 1.0     # m2 (src) pair p -> rows 2p..
        bd[64:128, 72 + p * 36 + 2 * p + 1] = 1.0
    ohp = np.zeros((36, 512), np.float32)
    for p in range(2):
        ohp[32 + 2 * p, p * 128:p * 128 + 64] = 1.0       # sink selectors
        ohp[32 + 2 * p + 1, p * 128 + 64:(p + 1) * 128] = 1.0
        ohp[2 * p, 256 + p * 128:256 + p * 128 + 64] = 1.0  # src selectors
        ohp[2 * p + 1, 256 + p * 128 + 64:256 + (p + 1) * 128] = 1.0
    normal = np.arange(1, L + 1, dtype=np.float32)
    nrm = np.broadcast_to(normal, (36, L)).copy()
    inrm = np.broadcast_to(1.0 / normal, (36, L)).copy()
    bf = lambda a: a.astype(ml_dtypes.bfloat16)
    return dict(triu=bf(np.tile(triu, (1, 4))), ident=bf(ident), bd=bf(bd),
                ohp=bf(ohp), nrm=nrm, inrm=inrm)


def kernel(**inputs):
    x = np.asarray(inputs["inputs"], np.float32)
    cst = _consts()
    bf = lambda a: np.ascontiguousarray(a, np.float32).astype(ml_dtypes.bfloat16)
    w1b = bf(inputs["W1"])
    w2b = bf(inputs["W2"])
    in_maps = []
    for c in range(8):
        b, g = c // 4, c % 4
        cols = slice(g * DC, (g + 1) * DC)
        t0 = slice(g * 128, (g + 1) * 128)
        t1 = slice(512 + g * 128, 512 + (g + 1) * 128)
        m = {
            "xT": bf(x[b].T),
            "wq": bf(np.asarray(inputs["Wq"])[:, cols]),
            "wk": bf(np.asarray(inputs["Wk"])[:, cols]),
            "wv": bf(np.asarray(inputs["Wv"])[:, cols]),
            "wo": bf(np.asarray(inputs["Wo"])[cols, :]),
            "w1": w1b, "w2": w2b,
            "xres": np.ascontiguousarray(
                np.concatenate([x[b, t0, :], x[b, t1, :]], axis=0), np.float32),
        }
        m.update({k: v.copy() for k, v in cst.items()})
        in_maps.append(m)

    if "nc" not in _CACHE:
        _CACHE["nc"] = build_program()
    global LAST_RESULT
    res = run_bass_kernel_spmd(_CACHE["nc"], in_maps, core_ids=list(range(8)),
                               trace=TRACE)
    LAST_RESULT = res
    out = np.zeros((B, L, D), np.float32)
    for c in range(8):
        b, g = c // 4, c % 4
        r = res.results[c]["out"]
        out[b, g * 128:(g + 1) * 128, :] = r[:128]
        out[b, 512 + g * 128:512 + (g + 1) * 128, :] = r[128:]
    return out


# revision 30
# speedup vs baseline: 1.0678x; 1.0678x over previous
"""Trainium2 Bass kernel for nn_DecoderLayer_90074054132191.

Sharding: 8 cores = 2 batch groups x 4 cores. Core c: batch b=c//4, head
group g=c%4 (heads 4g..4g+4), token slice g*256..(g+1)*256 after a
ReduceScatter of the partial attention output over the 4-core group.

Phase A is software-pipelined over the two 512-token halves: each gate
stage runs for half h0 as soon as its inputs exist, so the first
ReduceScatter fires while half h1 is still in the gate pipeline, and the
FFN (phase B, split per 128-token output block) overlaps the second
ReduceScatter.
"""

import math
from contextlib import ExitStack

import ml_dtypes
import numpy as np

import concourse.bass as bass
import concourse.mybir as mybir
import concourse.tile as tile
from concourse import bacc
from concourse.bass_utils import run_bass_kernel_spmd

F32 = mybir.dt.float32
BF16 = mybir.dt.bfloat16
AF = mybir.ActivationFunctionType
OP = mybir.AluOpType

B, L, D, H, F = 2, 1024, 1024, 16, 4096
DH = 64          # head dim
HPC = 4          # heads per core
DC = HPC * DH    # 256 features per core
TOK = 256        # tokens per core after reduce-scatter
C = 128          # chunk size
NCH = L // C     # 8 chunks
NKT = D // 128   # 8 k-tiles of the model dim


def build_program(sim_nocc=False):
    nc = bacc.Bacc("TRN2", target_bir_lowering=False, num_devices=8)

    # ---- external I/O (per-core shards prepared on host) ----
    xT = nc.dram_tensor("xT", [D, L], BF16, kind="ExternalInput")
    wq = nc.dram_tensor("wq", [D, DC], BF16, kind="ExternalInput")
    wk = nc.dram_tensor("wk", [D, DC], BF16, kind="ExternalInput")
    wv = nc.dram_tensor("wv", [D, DC], BF16, kind="ExternalInput")
    wo = nc.dram_tensor("wo", [DC, D], BF16, kind="ExternalInput")
    w1 = nc.dram_tensor("w1", [D, F], BF16, kind="ExternalInput")
    w2 = nc.dram_tensor("w2", [F, D], BF16, kind="ExternalInput")
    xres = nc.dram_tensor("xres", [TOK, D], F32, kind="ExternalInput")
    triu_c = nc.dram_tensor("triu", [C, 4 * C], BF16, kind="ExternalInput")
    ident_c = nc.dram_tensor("ident", [128, 128], BF16, kind="ExternalInput")
    bd_c = nc.dram_tensor("bd", [128, 144], BF16, kind="ExternalInput")
    ohp_c = nc.dram_tensor("ohp", [36, 512], BF16, kind="ExternalInput")
    nrm_c = nc.dram_tensor("nrm", [36, L], F32, kind="ExternalInput")
    inrm_c = nc.dram_tensor("inrm", [36, L], F32, kind="ExternalInput")
    out = nc.dram_tensor("out", [TOK, D], F32, kind="ExternalOutput")

    with ExitStack() as top:
        tc = top.enter_context(tile.TileContext(nc))
        consts = top.enter_context(tc.tile_pool(name="consts", bufs=1))
        dram = top.enter_context(tc.tile_pool(name="dram", bufs=1, space="DRAM"))

        rs_in = dram.tile([L, D], BF16)
        rs_out0 = dram.tile([TOK // 2, D], BF16)
        rs_out1 = dram.tile([TOK // 2, D], BF16)

        # ---- consts to SBUF ----
        triu = consts.tile([C, 4 * C], BF16)
        ident = consts.tile([128, 128], BF16)
        bd = consts.tile([128, 144], BF16)
        ohp = consts.tile([36, 512], BF16)
        nrm = consts.tile([36, L], F32)
        inrm = consts.tile([36, L], F32)
        epsb = consts.tile([128, 1], F32)
        nc.vector.memset(epsb, 1e-5)
        scr1 = consts.tile([1, 1], F32)  # ACT-table preload scratch

        w1pool = top.enter_context(tc.tile_pool(name="w1pool", bufs=1))
        w1_sb = []
        early = top.enter_context(tc.tile_pool(name="early", bufs=1))
        hT_sb = []
        hn0 = None
        with ExitStack() as pa:  # ---------------- PHASE A ----------------
            awork = pa.enter_context(tc.tile_pool(name="awork", bufs=1))
            pbig = pa.enter_context(
                tc.tile_pool(name="pbig", bufs=2, space="PSUM"))
            psmall = pa.enter_context(
                tc.tile_pool(name="psmall", bufs=2, space="PSUM"))
            pyp = pa.enter_context(tc.tile_pool(name="pyp", bufs=2, space="PSUM"))
            pkv = pa.enter_context(tc.tile_pool(name="pkv", bufs=1, space="PSUM"))

            xw = ExitStack()
            xwpool = xw.enter_context(tc.tile_pool(name="xwpool", bufs=1))
            xT_sb, wq_sb, wk_sb, wv_sb = [], [], [], []
            for kt in range(NKT):
                t = xwpool.tile([128, L], BF16, name=f"xT{kt}")
                nc.sync.dma_start(t[:, 0:512], xT[kt * 128:(kt + 1) * 128, 0:512])
                xT_sb.append(t)
                t2 = xwpool.tile([128, DC], BF16, name=f"wk{kt}")
                nc.sync.dma_start(t2, wk[kt * 128:(kt + 1) * 128, :])
                wk_sb.append(t2)
            for kt in range(NKT):
                t2 = xwpool.tile([128, DC], BF16, name=f"wq{kt}")
                nc.sync.dma_start(t2, wq[kt * 128:(kt + 1) * 128, :])
                wq_sb.append(t2)
            last_load = None
            for kt in range(NKT):
                nc.sync.dma_start(xT_sb[kt][:, 512:1024],
                                  xT[kt * 128:(kt + 1) * 128, 512:1024])
                t2 = xwpool.tile([128, DC], BF16, name=f"wv{kt}")
                last_load = nc.sync.dma_start(t2, wv[kt * 128:(kt + 1) * 128, :])
                wv_sb.append(t2)
            nc.sync.dma_start(bd, bd_c[:])
            nc.sync.dma_start(nrm, nrm_c[:])
            nc.sync.dma_start(inrm, inrm_c[:])
            nc.sync.dma_start(triu, triu_c[:])
            nc.sync.dma_start(ident, ident_c[:])
            nc.sync.dma_start(ohp, ohp_c[:])
            wo_sb = []
            for p in range(2):
                t = awork.tile([128, D], BF16, name=f"wo{p}")
                nc.sync.dma_start(t, wo[p * 128:(p + 1) * 128, :])
                wo_sb.append(t)
            # deferred weight prefetches (behind the projection loads)
            for kt in range(NKT):
                t = w1pool.tile([128, F], BF16, name=f"w1{kt}")
                d = nc.sync.dma_start(t, w1[kt * 128:(kt + 1) * 128, :])
                tile.add_dep_helper(d.ins, last_load.ins, sync=False,
                                    reason="defer w1 prefetch")
                w1_sb.append(t)

            # ---- persistent tiles ----
            Q = [awork.tile([128, L], BF16, name=f"q{p}") for p in range(2)]
            K = [awork.tile([128, L], BF16, name=f"k{p}") for p in range(2)]
            V = [awork.tile([128, L], BF16, name=f"v{p}") for p in range(2)]
            # scan outputs (also reused for stage-2 scans)
            ck = [awork.tile([128, L], BF16, name=f"ck{p}") for p in range(2)]
            cq = [awork.tile([128, L], BF16, name=f"cq{p}") for p in range(2)]
            # mul products feeding the bd matmuls
            m1 = [awork.tile([128, L], BF16, name=f"m1{p}") for p in range(2)]
            m2 = [awork.tile([128, L], BF16, name=f"m2{p}") for p in range(2)]
            # stage-2 products
            wk2 = [awork.tile([128, L], BF16, name=f"wk2{p}") for p in range(2)]
            wq2 = [awork.tile([128, L], BF16, name=f"wq2{p}") for p in range(2)]
            DEN = awork.tile([36, L], F32)
            IDEN = awork.tile([36, L], F32)
            S = awork.tile([36, L], BF16)
            CONS = awork.tile([36, L], F32)
            R2 = awork.tile([36, L], BF16)
            nc.gpsimd.memset(R2, 0.0)
            KT_sb = [awork.tile([128, L], BF16, name=f"ktok{p}")
                     for p in range(2)]
            VT_sb = [awork.tile([128, L], BF16, name=f"vtok{p}")
                     for p in range(2)]
            qfacB = [awork.tile([128, L], BF16, name=f"qfacB{p}")
                     for p in range(2)]
            attnT = [awork.tile([128, L], BF16, name=f"attnT{p}")
                     for p in range(2)]
            kv_sb = [awork.tile([128, DH], BF16, name=f"kv{p}")
                     for p in range(2)]
            kvps = [pkv.tile([128, DH], F32, name=f"kvps{p}", tag=f"kvp{p}")
                    for p in range(2)]
            M2 = {}

            def proj(h):
                sl = slice(h * 512, (h + 1) * 512)
                for nm, wsb, dst in (("k", wk_sb, K), ("q", wq_sb, Q),
                                     ("v", wv_sb, V)):
                    for mt in range(2):
                        pool, tg = [(pbig, "big"), (pyp, "y"),
                                    (psmall, "sm")][mt % 3]
                        ps = pool.tile([128, 512], F32, tag=tg)
                        for kt in range(NKT):
                            nc.tensor.matmul(
                                ps,
                                wsb[kt][:, mt * 128:(mt + 1) * 128],
                                xT_sb[kt][:, sl],
                                start=(kt == 0), stop=(kt == NKT - 1))
                        dsl = dst[mt][:, sl]
                        if nm == "v":
                            nc.scalar.copy(out=dsl, in_=ps)
                        else:
                            nc.scalar.activation(dsl, ps, AF.Sigmoid)

            def scan1(h):
                sl = slice(h * 512, (h + 1) * 512)
                for p in range(2):
                    ini_k = 0.0 if h == 0 else ck[p][:, 511:512]
                    ini_q = 0.0 if h == 0 else cq[p][:, 511:512]
                    nc.vector.tensor_tensor_scan(ck[p][:, sl], K[p][:, sl],
                                                 K[p][:, sl], ini_k,
                                                 OP.add, OP.bypass)
                    nc.vector.tensor_tensor_scan(cq[p][:, sl], Q[p][:, sl],
                                                 Q[p][:, sl], ini_q,
                                                 OP.add, OP.bypass)

            def mmul(h):  # m1 = Q*cumK, m2 = K*cumQ
                sl = slice(h * 512, (h + 1) * 512)
                for p in range(2):
                    eng = nc.vector if p == 0 else nc.gpsimd
                    eng.tensor_mul(out=m1[p][:, sl], in0=Q[p][:, sl],
                                   in1=ck[p][:, sl])
                    eng.tensor_mul(out=m2[p][:, sl], in0=K[p][:, sl],
                                   in1=cq[p][:, sl])

            def bdmm(h, srcs, dst_f32, via):
                sl = slice(h * 512, (h + 1) * 512)
                ps = pbig.tile([36, 512], F32, tag="big")
                nc.tensor.matmul(ps, bd[:, 0:36], srcs[0][0][:, sl],
                                 start=True, stop=False)
                nc.tensor.matmul(ps, bd[:, 36:72], srcs[0][1][:, sl],
                                 start=False, stop=False)
                nc.tensor.matmul(ps, bd[:, 72:108], srcs[1][0][:, sl],
                                 start=False, stop=False)
                nc.tensor.matmul(ps, bd[:, 108:144], srcs[1][1][:, sl],
                                 start=False, stop=True)
                if via == "vadd":
                    # +1e-30 keeps unused zero rows finite through reciprocal
                    nc.vector.tensor_scalar_add(out=dst_f32[:, sl], in0=ps,
                                                scalar1=1e-30)
                else:
                    nc.scalar.copy(out=dst_f32[:, sl], in_=ps)

            def ktm2(h):
                for p in range(2):
                    for ci in range(4):
                        c = h * 4 + ci
                        sl = slice(c * 128, (c + 1) * 128)
                        pst = psmall.tile([128, 128], BF16, tag="sm")
                        nc.tensor.transpose(pst, K[p][:, sl], ident)
                        nc.scalar.copy(out=KT_sb[p][:, sl], in_=pst)
                # packed masked diag blocks: one [128,512] psum per (p,hh)
                for p in range(2):
                    for hh in range(2):
                        rows = slice(hh * 64, (hh + 1) * 64)
                        aps = pyp.tile([128, 512], F32, tag="y")
                        for ci in range(4):
                            c = h * 4 + ci
                            sl = slice(c * 128, (c + 1) * 128)
                            nc.tensor.matmul(aps[:, ci * 128:(ci + 1) * 128],
                                             K[p][rows, sl], Q[p][rows, sl],
                                             start=True, stop=True)
                        m = awork.tile([128, 512], BF16,
                                       name=f"m2_{p}_{h}_{hh}")
                        nc.vector.tensor_mul(out=m, in0=aps, in1=triu)
                        for ci in range(4):
                            M2[(p, h * 4 + ci, hh)] = \
                                m[:, ci * 128:(ci + 1) * 128]

            def denpost(h):
                sl = slice(h * 512, (h + 1) * 512)
                nc.vector.reciprocal_approx_fast(out=IDEN[:, sl],
                                                 in_=DEN[:, sl])
                nc.vector.tensor_mul(out=S[:, sl], in0=nrm[:, sl],
                                     in1=IDEN[:, sl])

            def ohps(h):  # broadcast S rows; wk2 = K*src_out, wq2 = Q*sink_in
                sl = slice(h * 512, (h + 1) * 512)
                for p in range(2):
                    ps1 = pbig.tile([128, 512], F32, tag="big")
                    ps2 = pbig.tile([128, 512], F32, tag="big")
                    nc.tensor.matmul(ps1, ohp[:, 256 + p * 128:256 + (p + 1) * 128],
                                     S[:, sl], start=True, stop=True)
                    nc.tensor.matmul(ps2, ohp[:, p * 128:(p + 1) * 128],
                                     S[:, sl], start=True, stop=True)
                    nc.vector.tensor_mul(out=wk2[p][:, sl], in0=K[p][:, sl],
                                         in1=ps1)
                    nc.vector.tensor_mul(out=wq2[p][:, sl], in0=Q[p][:, sl],
                                         in1=ps2)

            def scan2(h):  # cumsum of wk2/wq2 into ck/cq (scan1 values dead)
                sl = slice(h * 512, (h + 1) * 512)
                for p in range(2):
                    ini_k = 0.0 if h == 0 else ck[p][:, 511:512]
                    ini_q = 0.0 if h == 0 else cq[p][:, 511:512]
                    nc.vector.tensor_tensor_scan(ck[p][:, sl], wk2[p][:, sl],
                                                 wk2[p][:, sl], ini_k,
                                                 OP.add, OP.bypass)
                    nc.vector.tensor_tensor_scan(cq[p][:, sl], wq2[p][:, sl],
                                                 wq2[p][:, sl], ini_q,
                                                 OP.add, OP.bypass)

            def mmul2(h):  # m1 = Q*cum(K*src), m2 = K*cum(Q*sink)
                sl = slice(h * 512, (h + 1) * 512)
                for p in range(2):
                    eng = nc.vector if p == 0 else nc.gpsimd
                    eng.tensor_mul(out=m1[p][:, sl], in0=Q[p][:, sl],
                                   in1=ck[p][:, sl])
                    eng.tensor_mul(out=m2[p][:, sl], in0=K[p][:, sl],
                                   in1=cq[p][:, sl])

            def conspost(h):
                sl = slice(h * 512, (h + 1) * 512)
                nc.vector.tensor_mul(out=CONS[:, sl], in0=CONS[:, sl],
                                     in1=inrm[:, sl])
                nc.vector.tensor_scalar(out=CONS[0:4, sl], in0=CONS[0:4, sl],
                                        scalar1=1.0, scalar2=-1.0,
                                        op0=OP.min, op1=OP.max)
                EX = S  # reuse
                nc.scalar.activation(EX[32:36, sl], CONS[32:36, sl], AF.Sigmoid)
                nc.vector.tensor_mul(out=R2[32:36, sl], in0=IDEN[32:36, sl],
                                     in1=EX[32:36, sl])
                nc.scalar.activation(EX[0:4, sl], CONS[0:4, sl], AF.Exp)
                CE = CONS
                ini = 0.0 if h == 0 else CE[0:4, 511:512]
                nc.vector.tensor_tensor_scan(CE[0:4, sl], EX[0:4, sl],
                                             EX[0:4, sl], ini,
                                             OP.add, OP.bypass)
                nc.vector.reciprocal_approx_fast(out=IDEN[0:4, sl],
                                                 in_=CE[0:4, sl])
                nc.vector.tensor_mul(out=EX[0:4, sl], in0=EX[0:4, sl],
                                     in1=IDEN[0:4, sl])
                nc.vector.tensor_mul(out=R2[0:4, sl], in0=EX[0:4, sl],
                                     in1=nrm[0:4, sl])

            def qfac(h):
                sl = slice(h * 512, (h + 1) * 512)
                for p in range(2):
                    ps1 = pbig.tile([128, 512], F32, tag="big")
                    ps2 = pbig.tile([128, 512], F32, tag="big")
                    nc.tensor.matmul(ps1, ohp[:, p * 128:(p + 1) * 128],
                                     R2[:, sl], start=True, stop=True)
                    nc.tensor.matmul(ps2, ohp[:, 256 + p * 128:256 + (p + 1) * 128],
                                     R2[:, sl], start=True, stop=True)
                    nc.scalar.copy(out=qfacB[p][:, sl], in_=ps1)
                    nc.vector.tensor_mul(out=V[p][:, sl], in0=V[p][:, sl],
                                         in1=ps2)

            def vst(h):
                for p in range(2):
                    for ci in range(4):
                        c = h * 4 + ci
                        sl = slice(c * 128, (c + 1) * 128)
                        pst2 = psmall.tile([128, 128], BF16, tag="sm")
                        nc.tensor.transpose(pst2, V[p][:, sl], ident)
                        nc.scalar.copy(out=VT_sb[p][:, sl], in_=pst2)

            def attn(grp):
                yp = [pyp.tile([128, 512], F32, tag="y", name=f"yp{grp}_{p}")
                      for p in range(2)]
                for ci in range(4):
                    c = grp * 4 + ci
                    sl = slice(c * 128, (c + 1) * 128)
                    for p in range(2):
                        ysl = yp[p][:, ci * 128:(ci + 1) * 128]
                        for hh in range(2):
                            rows = slice(hh * 64, (hh + 1) * 64)
                            first = (c == 0)
                            if not first:
                                nc.tensor.matmul(ysl[rows, :], kv_sb[p][rows, :],
                                                 Q[p][rows, sl],
                                                 start=True, stop=False)
                            nc.tensor.matmul(
                                ysl[rows, :],
                                VT_sb[p][:, c * 128 + hh * 64:c * 128 + hh * 64 + 64],
                                M2[(p, c, hh)], start=first, stop=True)
                            nc.tensor.matmul(
                                kvps[p][rows, :],
                                KT_sb[p][:, c * 128 + hh * 64:c * 128 + hh * 64 + 64],
                                VT_sb[p][:, c * 128 + hh * 64:c * 128 + hh * 64 + 64],
                                start=first, stop=(c == NCH - 1))
                    if c < NCH - 1:
                        for p in range(2):
                            nc.vector.tensor_copy(out=kv_sb[p], in_=kvps[p])
                for p in range(2):
                    nc.vector.tensor_mul(
                        out=attnT[p][:, grp * 512:(grp + 1) * 512],
                        in0=qfacB[p][:, grp * 512:(grp + 1) * 512], in1=yp[p])

            def womm(grp):
                for tci in range(4):
                    tch = grp * 4 + tci
                    wo_out = awork.tile([128, D], BF16, tag="wo_out", bufs=2)
                    for nt in range(2):
                        ps = pbig.tile([128, 512], F32, tag="big")
                        for p in range(2):
                            nc.tensor.matmul(
                                ps, attnT[p][:, tch * 128:(tch + 1) * 128],
                                wo_sb[p][:, nt * 512:(nt + 1) * 512],
                                start=(p == 0), stop=(p == 1))
                        osl = wo_out[:, nt * 512:(nt + 1) * 512]
                        if nt == 0:
                            nc.scalar.copy(out=osl, in_=ps)
                        else:
                            nc.vector.tensor_copy(out=osl, in_=ps)
                    nc.sync.dma_start(rs_in[tch * 128:(tch + 1) * 128, :], wo_out)

            def rstrig(grp):
                rs_half = rs_out0 if grp == 0 else rs_out1
                if sim_nocc:
                    nc.sync.dma_start(rs_half[:, :],
                                      rs_in[grp * 512:grp * 512 + TOK // 2, :])
                else:
                    nc.gpsimd.collective_compute(
                        "ReduceScatter", OP.add,
                        replica_groups=[[0, 1, 2, 3], [4, 5, 6, 7]],
                        ins=[rs_in[grp * 512:(grp + 1) * 512, :].opt()],
                        outs=[rs_half.opt()])

            # ---------- pipelined emission ----------
            proj(0)                 # PE: h0 projections
            proj(1)                 # PE: h1 projections
            xw.close()              # free xT + qkv weight SBUF
            scan1(0)                # DVE p0 / GpSimd p1
            mmul(0)                 # DVE p0 / GpSimd p1
            bdmm(0, (m1, m2), DEN, "vadd")       # PE + scalar copy(+1e-30)
            # preload the exp ACT table while the PE runs the DEN matmuls
            nc.scalar.activation(scr1, epsb[0:1, 0:1], AF.Exp)
            denpost(0)              # DVE recip + S
            scan1(1)
            ktm2(0)                 # PE transposes + packed M2 + DVE muls
            ohps(0)                 # PE + DVE
            mmul(1)
            scan2(0)
            bdmm(1, (m1, m2), DEN, "vadd")
            mmul2(0)
            denpost(1)
            bdmm(0, (m1, m2), CONS, "scopy")
            conspost(0)
            ohps(1)
            qfac(0)
            scan2(1)
            ktm2(1)
            vst(0)
            mmul2(1)
            attn(0)
            bdmm(1, (m1, m2), CONS, "scopy")
            womm(0)
            rstrig(0)
            conspost(1)
            qfac(1)
            vst(1)
            attn(1)
            womm(1)
            rstrig(1)
            # preload the sqrt ACT table before LN1 needs it
            nc.scalar.activation(scr1, epsb[0:1, 0:1], AF.Sqrt)

            # ---- tt=0 residual + LN1 + hT transposes (overlap RS#1) ----
            att0 = early.tile([128, D], BF16, name="att0")
            hh0 = early.tile([128, D], F32, name="hh0")
            att0_dma = nc.sync.dma_start(att0, rs_out0[:, :])
            nc.sync.dma_start(hh0, xres[0:128, :])
            nc.vector.tensor_add(out=hh0, in0=hh0, in1=att0)
            st0 = early.tile([128, 2, 6], F32, name="st0")
            mv0 = early.tile([128, 2], F32, name="mv0")
            for sg in range(2):
                nc.vector.bn_stats(out=st0[:, sg, :],
                                   in_=hh0[:, sg * 512:(sg + 1) * 512])
            nc.vector.bn_aggr(out=mv0, in_=st0)
            sd0 = early.tile([128, 1], F32, name="sd0")
            rstd0 = early.tile([128, 1], F32, name="rstd0")
            nc.scalar.activation(sd0, mv0[:, 1:2], AF.Sqrt, bias=epsb)
            nc.vector.reciprocal(out=rstd0, in_=sd0)
            hn0 = early.tile([128, D], BF16, name="hn0")
            nc.vector.tensor_scalar(out=hn0, in0=hh0, scalar1=mv0[:, 0:1],
                                    scalar2=rstd0, op0=OP.subtract,
                                    op1=OP.mult)
            for kt in range(NKT):
                t = early.tile([128, TOK], BF16, name=f"hT{kt}")
                hT_sb.append(t)
                pst = psmall.tile([128, 128], BF16, tag="sm")
                nc.tensor.transpose(pst, hn0[:, kt * 128:(kt + 1) * 128],
                                    ident)
                if kt % 2 == 0:
                    nc.vector.tensor_copy(out=t[:, 0:128], in_=pst)
                else:
                    nc.scalar.copy(out=t[:, 0:128], in_=pst)

        with ExitStack() as pb:  # ---------------- PHASE B ----------------
            bwork = pb.enter_context(tc.tile_pool(name="bwork", bufs=1))
            w2pool = pb.enter_context(tc.tile_pool(name="w2pool", bufs=1))
            pb1 = pb.enter_context(tc.tile_pool(name="pb1", bufs=4, space="PSUM"))
            pb2 = pb.enter_context(tc.tile_pool(name="pb2", bufs=1, space="PSUM"))

            # W2 prefetch: deferred until RS#0 completes so the collective
            # doesn't compete with an 8MB weight stream
            w2_sb = []
            for kt2 in range(F // 128):
                wt = w2pool.tile([128, D], BF16, name=f"w2{kt2}")
                d = nc.sync.dma_start(wt, w2[kt2 * 128:(kt2 + 1) * 128, :])
                tile.add_dep_helper(d.ins, att0_dma.ins, sync=False,
                                    reason="defer w2 behind RS#0")
                w2_sb.append(wt)
            att1 = bwork.tile([128, D], BF16, name="att1")
            xr1 = bwork.tile([128, D], F32, name="xr1")
            nc.sync.dma_start(att1, rs_out1[:, :])
            nc.sync.dma_start(xr1, xres[128:256, :])

            hn = [hn0, None]
            gT = [[], []]
            for tt in range(2):
                if tt == 1:
                    hh_t = bwork.tile([128, D], F32, name="hh1")
                    nc.vector.tensor_add(out=hh_t, in0=att1, in1=xr1)
                    # LN1
                    stats = bwork.tile([128, 2, 6], F32, tag="st", bufs=2)
                    mv = bwork.tile([128, 2], F32, tag="mv", bufs=2)
                    for sg in range(2):
                        nc.vector.bn_stats(out=stats[:, sg, :],
                                           in_=hh_t[:, sg * 512:(sg + 1) * 512])
                    nc.vector.bn_aggr(out=mv, in_=stats)
                    sd = bwork.tile([128, 1], F32, tag="sd", bufs=2)
                    rstd = bwork.tile([128, 1], F32, tag="rstd", bufs=2)
                    nc.scalar.activation(sd, mv[:, 1:2], AF.Sqrt, bias=epsb)
                    nc.vector.reciprocal(out=rstd, in_=sd)
                    hn_t = bwork.tile([128, D], BF16, name="hn1")
                    nc.vector.tensor_scalar(out=hn_t, in0=hh_t,
                                            scalar1=mv[:, 0:1], scalar2=rstd,
                                            op0=OP.subtract, op1=OP.mult)
                    hn[1] = hn_t
                    for kt in range(NKT):
                        pst = pb1.tile([128, 128], BF16, tag="pb1")
                        nc.tensor.transpose(pst, hn_t[:, kt * 128:(kt + 1) * 128],
                                            ident)
                        nc.vector.tensor_copy(
                            out=hT_sb[kt][:, 128:256], in_=pst)

                # W1 + gelu for this token half only (overlaps the other
                # half's ReduceScatter)
                csl = slice(tt * 128, (tt + 1) * 128)
                for mt in range(F // 128):
                    ps = pb1.tile([128, 128], F32, tag="pb1")
                    for kt in range(NKT):
                        nc.tensor.matmul(ps, w1_sb[kt][:, mt * 128:(mt + 1) * 128],
                                         hT_sb[kt][:, csl], start=(kt == 0),
                                         stop=(kt == NKT - 1))
                    g = bwork.tile([128, 128], BF16, name=f"g{tt}_{mt}")
                    nc.scalar.activation(g, ps, AF.Gelu)
                    gT[tt].append(g)

                y2ps = [pb2.tile([128, 512], F32, name=f"y2_{tt}_{nt}",
                                 tag=f"y2{nt}", bufs=2) for nt in range(2)]
                for kt2 in range(F // 128):
                    for nt in range(2):
                        nc.tensor.matmul(
                            y2ps[nt],
                            gT[tt][kt2],
                            w2_sb[kt2][:, nt * 512:(nt + 1) * 512],
                            start=(kt2 == 0), stop=(kt2 == F // 128 - 1))
                x2 = bwork.tile([128, D], F32, tag="x2", bufs=2)
                for nt in range(2):
                    nc.vector.tensor_add(
                        out=x2[:, nt * 512:(nt + 1) * 512],
                        in0=hn[tt][:, nt * 512:(nt + 1) * 512],
                        in1=y2ps[nt])
                stats = bwork.tile([128, 2, 6], F32, tag="st2", bufs=2)
                mv = bwork.tile([128, 2], F32, tag="mv2", bufs=2)
                for sg in range(2):
                    nc.vector.bn_stats(out=stats[:, sg, :],
                                       in_=x2[:, sg * 512:(sg + 1) * 512])
                nc.vector.bn_aggr(out=mv, in_=stats)
                sd2 = bwork.tile([128, 1], F32, tag="sd2", bufs=2)
                rstd = bwork.tile([128, 1], F32, tag="rstd2", bufs=2)
                nc.scalar.activation(sd2, mv[:, 1:2], AF.Sqrt, bias=epsb)
                nc.vector.reciprocal(out=rstd, in_=sd2)
                nc.vector.tensor_scalar(out=x2, in0=x2, scalar1=mv[:, 0:1],
                                        scalar2=rstd, op0=OP.subtract, op1=OP.mult)
                nc.sync.dma_start(out[tt * 128:(tt + 1) * 128, :], x2)

    nc.compile()
    return nc


_CACHE = {}
TRACE = False
LAST_RESULT = None


def _consts():
    triu = np.triu(np.ones((C, C), np.float32))
    ident = np.eye(128, dtype=np.float32)
    # row groups: src rows 0-3, sink rows 32-35 (legal partition bases)
    bd = np.zeros((128, 144), np.float32)
    for p in range(2):
        bd[0:64, p * 36 + 32 + 2 * p] = 1.0     # m1 (sink) pair p -> rows 32+
        bd[64:128, p * 36 + 32 + 2 * p + 1] = 1.0
        bd[0:64, 72 + p * 36 + 2 * p] = 1.0     # m2 (src) pair p -> rows 2p..
        bd[64:128, 72 + p * 36 + 2 * p + 1] = 1.0
    ohp = np.zeros((36, 512), np.float32)
    for p in range(2):
        ohp[32 + 2 * p, p * 128:p * 128 + 64] = 1.0       # sink selectors
        ohp[32 + 2 * p + 1, p * 128 + 64:(p + 1) * 128] = 1.0
        ohp[2 * p, 256 + p * 128:256 + p * 128 + 64] = 1.0  # src selectors
        ohp[2 * p + 1, 256 + p * 128 + 64:256 + (p + 1) * 128] = 1.0
    normal = np.arange(1, L + 1, dtype=np.float32)
    nrm = np.broadcast_to(normal, (36, L)).copy()
    inrm = np.broadcast_to(1.0 / normal, (36, L)).copy()
    bf = lambda a: a.astype(ml_dtypes.bfloat16)
    return dict(triu=bf(np.tile(triu, (1, 4))), ident=bf(ident), bd=bf(bd),
                ohp=bf(ohp), nrm=nrm, inrm=inrm)


def kernel(**inputs):
    x = np.asarray(inputs["inputs"], np.float32)
    cst = _consts()
    bf = lambda a: np.ascontiguousarray(a, np.float32).astype(ml_dtypes.bfloat16)
    w1b = bf(inputs["W1"])
    w2b = bf(inputs["W2"])
    in_maps = []
    for c in range(8):
        b, g = c // 4, c % 4
        cols = slice(g * DC, (g + 1) * DC)
        t0 = slice(g * 128, (g + 1) * 128)
        t1 = slice(512 + g * 128, 512 + (g + 1) * 128)
        m = {
            "xT": bf(x[b].T),
            "wq": bf(np.asarray(inputs["Wq"])[:, cols]),
            "wk": bf(np.asarray(inputs["Wk"])[:, cols]),
            "wv": bf(np.asarray(inputs["Wv"])[:, cols]),
            "wo": bf(np.asarray(inputs["Wo"])[cols, :]),
            "w1": w1b, "w2": w2b,
            "xres": np.ascontiguousarray(
                np.concatenate([x[b, t0, :], x[b, t1, :]], axis=0), np.float32),
        }
        m.update({k: v.copy() for k, v in cst.items()})
        in_maps.append(m)

    if "nc" not in _CACHE:
        _CACHE["nc"] = build_program()
    global LAST_RESULT
    res = run_bass_kernel_spmd(_CACHE["nc"], in_maps, core_ids=list(range(8)),
                               trace=TRACE)
    LAST_RESULT = res
    out = np.zeros((B, L, D), np.float32)
    for c in range(8):
        b, g = c // 4, c % 4
        r = res.results[c]["out"]
        out[b, g * 128:(g + 1) * 128, :] = r[:128]
        out[b, 512 + g * 128:512 + (g + 1) * 128, :] = r[128:]
    return out
